# revision 64
# baseline (speedup 1.0000x reference)
"""Trainium2 Bass kernel: multi-head attention (Graphormer-style bias+mask)
followed by a node-similarity GEMM (out = merged @ merged^T).

Sharding: pure data-parallel over batch. B=8 batch elements -> 8 NeuronCores,
one batch element per core, no collectives. Each core computes its own
[1024, 1024] output slab.

Design (transposed-scores layout; per-core, batch b fixed):
  Q^T = Wq x^T + bq                      [C, N] f32r (d on partitions)
  K^T = Wk x^T                           bk is softmax-shift-invariant: the
                                         b_k.Q[n] and b_k.b_q score terms are
                                         constant along the softmax axis, so
                                         dropping bk is mathematically exact
  V   = x Wv^T + bv                      [N, C] bf16, head-slices computed
                                         just in time (one slice per step)
  S^T[m,n] = K Q^T                       per (head, m-tile): lhsT=K^T-slice,
                                         rhs=Q^T -> PSUM [128, N]; no bias or
                                         mask work on the PE at all
  E0  = exp(S^T/8)                       ACT, psum -> sbuf bf16 (the pacing
                                         engine: 64 x [128,1024] exps)
  E^T = E0 * B^T[h]                      DVE 2x / Pool; B = exp((bias+mneg)/8)
                                         folded on the host (exp(a+b) =
                                         exp(a)exp(b)), so masked entries are
                                         exactly 0 and no mask add is needed
  A[n,dslice], rs[n] = E^T-blocks @ V    A-natural matmuls: lhsT=E^T-block
                                         (m on partitions), rhs=V-slice/ones;
                                         rowsums ride along as a free column
  merged[n, h*64:..] = A * (1/rs)        DVE normalize fused into the
                                         PSUM->SBUF copy
  mergedT = transpose(merged)            PE bf16 transposes per head-pair
  out = mergedT^T @ mergedT              f32 PSUM, bf16 operands; blocks
                                         below the 128-row block diagonal are
                                         skipped (symmetry) and mirrored on
                                         the host

Schedule: a software pipeline interleaves, per (head, m-tile) step, the S^T
matmuls, the lagged phase-2 (A+normalize) groups of the previous head, the
JIT V slices, spread-out Q^T/K^T blocks, and merged transposes, so the PE
stream never blocks the ACT exp stream.  E^T double-buffers by head parity.
The final GEMM prefills its first PSUM accumulation groups (contractions over
finished channel blocks) inside the last head's phase-2 window.
"""

import sys

if "/opt/trn_rl_repo" not in sys.path:
    sys.path.insert(0, "/opt/trn_rl_repo")

import ml_dtypes
import numpy as np

P = 128
N = 1024
C = 512
H = 8
D = 64  # head dim
NT = N // P  # 8 row tiles
CT = C // P  # 4 channel tiles
NCORES = 8

_CACHE = {}


def _build_nc():
    import concourse.mybir as mybir
    import concourse.tile as tile
    from concourse import bacc
    from concourse.masks import make_identity

    f32 = mybir.dt.float32
    f32r = mybir.dt.float32r
    bf16 = mybir.dt.bfloat16
    Act = mybir.ActivationFunctionType

    nc = bacc.Bacc("TRN2", target_bir_lowering=False, debug=False)

    # ---- DRAM parameters (per-core) ----
    # wfirst rows = input channel; cols = [wq strip ct0 | wk strip ct0 | x^T]
    # wrest cols = [wq strips ct1-3 | wk strips ct1-3 | wv]  (all bf16)
    WF = 2 * P + N
    D65 = D + 1
    WR = 2 * (C - P) + H * D65
    wfirst_d = nc.dram_tensor("wfirst", [C, WF], bf16, kind="ExternalInput")
    wrest_d = nc.dram_tensor("wrest", [C, WR], bf16, kind="ExternalInput")
    bqk_d = nc.dram_tensor("bqk", [P, 2 * CT], f32, kind="ExternalInput")
    bv_d = nc.dram_tensor("bv", [1, H * (D + 1)], bf16, kind="ExternalInput")
    BT_d = nc.dram_tensor("BT", [H, N, N], bf16, kind="ExternalInput")
    out_d = nc.dram_tensor("out", [N, N], bf16, kind="ExternalOutput")

    with tile.TileContext(nc) as tc:
        with (
            tc.tile_pool(name="const", bufs=1) as constp,
            tc.tile_pool(name="pers", bufs=1) as pers,
            tc.tile_pool(name="stream", bufs=2) as stream,
            tc.tile_pool(name="psS", bufs=2, space="PSUM") as psS,
            tc.tile_pool(name="psA", bufs=2, space="PSUM") as psA,
            tc.tile_pool(name="psT", bufs=2, space="PSUM") as psT,
        ):
            ident = constp.tile([P, P], f32)
            make_identity(nc, ident[:])
            ident_b = constp.tile([P, P], bf16)
            nc.vector.tensor_copy(ident_b[:], ident[:])

            warm = constp.tile([P, 1], f32)
            nc.scalar.activation(warm[:], ident[:, 0:1], Act.Exp, scale=1.0)
            # touch the PE right away so its p-state ramp timer starts
            # counting from ~0 and the first QK matmuls run at full clock
            warm_ps = psA.tile([1, 1], f32, tag="A", name="warm_ps")
            nc.tensor.matmul(
                warm_ps[:], ident[:, 0:1], ident[:, 0:1], start=True, stop=True
            )

            # ---- persistent SBUF tensors ----
            QT = [pers.tile([P, N], f32r, name=f"QT{i}") for i in range(CT)]
            KT = [pers.tile([P, N], f32r, name=f"KT{i}") for i in range(CT)]
            V = [pers.tile([P, H * D65], bf16, name=f"V{i}") for i in range(NT)]
            # E^T tiles, double-buffered by head parity: [slot][m-tile]
            ET = [
                [pers.tile([P, N], bf16, name=f"ET{s}_{i}") for i in range(NT)]
                for s in range(2)
            ]
            merged = [pers.tile([P, C], bf16, name=f"merged{i}") for i in range(NT)]
            mergedT = [pers.tile([P, N], bf16, name=f"mergedT{i}") for i in range(CT)]
            bqk_sb = pers.tile([P, 2 * CT], f32, name="bqk_sb")
            bv_sb = pers.tile([1, H * D65], bf16, name="bv_sb")
            ones_b = pers.tile([1, N], bf16, name="ones_b")
            wfirst = [pers.tile([P, WF], bf16, name=f"wfirst{i}") for i in range(CT)]
            wrest = [pers.tile([P, WR], bf16, name=f"wrest{i}") for i in range(CT)]
            xTb = [wfirst[i][:, 2 * P : WF] for i in range(CT)]
            wv = [wrest[i][:, 2 * (C - P) : WR] for i in range(CT)]

            def wq_strip(kt, ct):
                if ct == 0:
                    return wfirst[kt][:, 0:P]
                return wrest[kt][:, (ct - 1) * P : ct * P]

            def wk_strip(kt, ct):
                if ct == 0:
                    return wfirst[kt][:, P : 2 * P]
                return wrest[kt][:, (C - P) + (ct - 1) * P : (C - P) + ct * P]

            SPLIT = 2 * P + 512  # [wq0 | wk0 | x cols 0:512]
            for i in range(CT):
                nc.sync.dma_start(
                    out=wfirst[i][:, 0:SPLIT],
                    in_=wfirst_d[i * P : (i + 1) * P, 0:SPLIT],
                )
            nc.sync.dma_start(out=bqk_sb[:], in_=bqk_d[:])

            def qk_chunk(ct, unit):
                """One (w, j) quarter of Q^T/K^T rows ct*128..: 4 matmuls."""
                strip, dst, boff = ((wq_strip, QT, 0), (wk_strip, KT, CT))[unit // 2]
                j = unit % 2
                ps = psT.tile([P, 512], f32, tag="T", name=f"qk{ct}{unit}")
                for kt in range(CT):
                    nc.tensor.matmul(
                        ps[:],
                        strip(kt, ct),
                        xTb[kt][:, j * 512 : (j + 1) * 512],
                        start=(kt == 0),
                        stop=(kt == CT - 1),
                    )
                if unit // 2 == 0:
                    nc.vector.tensor_scalar_add(
                        dst[ct][:, j * 512 : (j + 1) * 512],
                        ps[:],
                        bqk_sb[:, boff + ct : boff + ct + 1],
                    )
                else:
                    # bk is softmax-shift-invariant -> plain copy
                    nc.vector.tensor_copy(
                        dst[ct][:, j * 512 : (j + 1) * 512], ps[:]
                    )

            # Q^T/K^T block 0, j half at a time, kt round-robin so the
            # matmuls track the wfirst DMA arrivals; the first S^T tile and
            # its exp run per half so ACT starts as early as possible.
            def qk_block0_half(j):
                pss = {}
                for unit in (j, 2 + j):
                    pss[unit] = psT.tile(
                        [P, 512], f32, tag="T", name=f"qk0{unit}"
                    )
                for kt in range(CT):
                    for unit in (j, 2 + j):
                        strip = (wq_strip, wk_strip)[unit // 2]
                        nc.tensor.matmul(
                            pss[unit][:],
                            strip(kt, 0),
                            xTb[kt][:, j * 512 : (j + 1) * 512],
                            start=(kt == 0),
                            stop=(kt == CT - 1),
                        )
                nc.vector.tensor_scalar_add(
                    QT[0][:, j * 512 : (j + 1) * 512],
                    pss[j][:],
                    bqk_sb[:, 0:1],
                )
                nc.vector.tensor_copy(
                    KT[0][:, j * 512 : (j + 1) * 512], pss[2 + j][:]
                )

            nc.vector.memset(ones_b[:], 1.0)

            def late_input_dmas():
                for i in range(CT):
                    nc.sync.dma_start(
                        out=wrest[i][:], in_=wrest_d[i * P : (i + 1) * P, :]
                    )
                nc.sync.dma_start(out=bv_sb[:], in_=bv_d[:])

            def v_slice(h, mt):
                # V65[mt][:, h*65:(h+1)*65] = (x Wv65^T + bv65) head-slice,
                # JIT.  wv65 col 64 of each slice is 0 and bv65 col 64 is 1,
                # so the slice carries a built-in ones column that later
                # yields the softmax row-sums for free in the A-matmul.
                ps = psA.tile([P, 72], f32, tag="A", name=f"vps{h}{mt}")
                for kt in range(CT):
                    nc.tensor.matmul(
                        ps[:, 0:D65],
                        xTb[kt][:, mt * P : (mt + 1) * P],
                        wv[kt][:, h * D65 : (h + 1) * D65],
                        start=(kt == 0),
                        stop=False,
                    )
                nc.tensor.matmul(
                    ps[:, 0:D65],
                    ones_b[:, mt * P : (mt + 1) * P],
                    bv_sb[:, h * D65 : (h + 1) * D65],
                    start=False,
                    stop=True,
                )
                nc.vector.tensor_copy(
                    V[mt][:, h * D65 : (h + 1) * D65], ps[:, 0:D65]
                )

            # ---- main loop over heads (software-pipelined) ----
            # Iteration h emits phase 1 (S^T -> E^T) of head h interleaved
            # with phase 2 (A-natural + normalize) of head h-1, so the PE
            # always has ready work while ACT chews through the exps.
            st_tiles = {}

            def st_tile(h, mt):
                qt = QT[h // 2]
                kt_sb = KT[h // 2]
                po = (h % 2) * D
                bt = stream.tile([P, N], bf16, tag="bt", bufs=6, name=f"bt{h}{mt}")
                nc.sync.dma_start(out=bt[:], in_=BT_d[h, mt * P : (mt + 1) * P, :])
                ST = psS.tile([P, N], f32, tag="S", name=f"ST{h}{mt}")
                for j in range(2):
                    nc.tensor.matmul(
                        ST[:, j * 512 : (j + 1) * 512],
                        kt_sb[po : po + D, mt * P : (mt + 1) * P],
                        qt[po : po + D, j * 512 : (j + 1) * 512],
                        start=True,
                        stop=True,
                    )
                st_tiles[(h, mt)] = (ST, bt)

            def ex_tile(h, mt, split=False):
                ST, bt = st_tiles.pop((h, mt))
                s = h % 2
                e0 = stream.tile([P, N], bf16, tag="e0", bufs=4, name=f"e0{h}{mt}")
                if split:
                    # first tile: exp each half as soon as its S^T half is
                    # ready, so ACT starts ~1.7us earlier
                    for j in range(2):
                        nc.scalar.activation(
                            e0[:, j * 512 : (j + 1) * 512],
                            ST[:, j * 512 : (j + 1) * 512],
                            Act.Exp,
                            scale=0.125,
                        )
                else:
                    nc.scalar.activation(e0[:], ST[:], Act.Exp, scale=0.125)
                # E^T = E0 * B^T (masked entries have B == 0); all-bf16
                # packed operands -> DVE 2x_1p mode.  Three tiles per head
                # go to the otherwise-idle Pool engine to unload DVE.
                eng = nc.gpsimd if mt in (0, 3, 6) else nc.vector
                eng.tensor_mul(ET[s][mt][:], e0[:], bt[:])

            def phase2_group(h, i, pool=None, trailing=False):
                s = h % 2
                Aps = (pool or psA).tile(
                    [P, 72], f32, tag="A" if pool is None else "T", name=f"A{h}{i}"
                )
                for mt in range(NT):
                    nc.tensor.matmul(
                        Aps[:, 0:D65],
                        ET[s][mt][:, i * P : (i + 1) * P],
                        V[mt][:, h * D65 : (h + 1) * D65],
                        start=(mt == 0),
                        stop=(mt == NT - 1),
                    )
                # merged = A * (1/rowsum)  (normalize while copying out)
                rc = stream.tile([P, 1], f32, tag="rc", bufs=4, name=f"rc{h}{i}")
                nc.vector.reciprocal(rc[:], Aps[:, D : D + 1])
                nc.vector.tensor_scalar_mul(
                    merged[i][:, h * D : (h + 1) * D], Aps[:, 0:D], rc[:]
                )

            def merged_transposes(ct, halves=(0, 1)):
                for half in halves:
                    tp = psT.tile([P, 512], bf16, tag="T", name=f"tp{ct}{half}")
                    for q in range(4):
                        i = half * 4 + q
                        nc.tensor.transpose(
                            tp[:, q * P : (q + 1) * P],
                            merged[i][:, ct * P : (ct + 1) * P],
                            ident_b[:],
                        )
                    nc.vector.tensor_copy(
                        mergedT[ct][:, half * 512 : (half + 1) * 512], tp[:]
                    )

            # Main loop: per (head, m-tile) step emit the S^T matmuls
            # first, then the lagged phase-2 group of the previous head,
            # then exp/B-mult (ACT only ever waits on the S^T matmuls,
            # which execute before the phase-2 burst), then side jobs.
            # hand-emitted warm-up: head 0 tile 0.  Both qk half-blocks are
            # emitted before the first S^T matmul so the half-1 matmuls can
            # track the x-j1 DMA arrivals instead of head-of-line blocking
            # behind ST00-j0 (which waits on the DVE bias add).
            qk_block0_half(0)
            bt00 = stream.tile([P, N], bf16, tag="bt", bufs=6, name="bt00")
            nc.sync.dma_start(out=bt00[:], in_=BT_d[0, 0:P, :])
            ST00 = psS.tile([P, N], f32, tag="S", name="ST00")
            e000 = stream.tile([P, N], bf16, tag="e0", bufs=4, name="e000")
            for i in range(CT):
                nc.sync.dma_start(
                    out=wfirst[i][:, SPLIT:WF],
                    in_=wfirst_d[i * P : (i + 1) * P, SPLIT:WF],
                )
            qk_block0_half(1)
            for j in range(2):
                nc.tensor.matmul(
                    ST00[:, j * 512 : (j + 1) * 512],
                    KT[0][0:D, 0:P],
                    QT[0][0:D, j * 512 : (j + 1) * 512],
                    start=True,
                    stop=True,
                )
                nc.scalar.activation(
                    e000[:, j * 512 : (j + 1) * 512],
                    ST00[:, j * 512 : (j + 1) * 512],
                    Act.Exp,
                    scale=0.125,
                )
            nc.vector.tensor_mul(ET[0][0][:], e000[:], bt00[:])

            for h in range(H):
                for mt in range(NT):
                    if h == 0 and mt == 0:
                        continue
                    st_tile(h, mt)
                    if h > 0 and mt >= 2:
                        # 2-tile lag so ET[h-1] is surely complete
                        phase2_group(h - 1, mt - 2)
                    if h > 0 and mt == 0:
                        phase2_group(h - 1, 6, trailing=True)
                    if h > 0 and mt == 1:
                        phase2_group(h - 1, 7, trailing=True)
                    if h >= 3 and h % 2 == 1 and mt == 2:
                        merged_transposes((h - 3) // 2)
                    ex_tile(h, mt)
                    if h == 0:
                        # head-0: late-input DMAs at mt 1, V slices after
                        # wrest lands, qk block 1 spread over mt 5..7
                        if mt == 1:
                            late_input_dmas()
                        if mt >= 4:
                            v_slice(0, 2 * (mt - 4))
                            v_slice(0, 2 * (mt - 4) + 1)
                        if mt >= 5:
                            qk_chunk(1, mt - 5)
                    else:
                        v_slice(h, mt)
                        # remaining Q^T/K^T chunks, at most two per head so
                        # no head's PE budget exceeds the ACT exp period
                        QK_SCHED = {
                            (1, 1): (1, 3),
                            (1, 5): (2, 0),
                            (2, 1): (2, 1),
                            (2, 5): (2, 2),
                            (3, 1): (2, 3),
                            (3, 5): (3, 0),
                            (4, 1): (3, 1),
                            (4, 5): (3, 2),
                            (5, 1): (3, 3),
                        }
                        if (h, mt) in QK_SCHED:
                            qk_chunk(*QK_SCHED[(h, mt)])


            def gemm_segments(i):
                # kept output columns for row-tile i: the diagonal and above
                # (out is symmetric; the host mirrors the rest)
                c0 = i * P
                return [(c0, 512), (512, 1024)] if c0 < 512 else [(c0, 1024)]

            def gemm_mms(i, half, cts):
                for (c0, c1), ps in zip(gemm_segments(i), half):
                    for ct in cts:
                        nc.tensor.matmul(
                            ps,
                            mergedT[ct][:, i * P : (i + 1) * P],
                            mergedT[ct][:, c0:c1],
                            start=(ct == 0),
                            stop=(ct == CT - 1),
                        )

            def gemm_out(i, half):
                # copies per segment (ACT/DVE alternating), then a single
                # DMA covering the whole kept column range (fewer HWDGE
                # round-trips in the drain)
                o_sb = stream.tile([P, N], bf16, tag="o_sb", bufs=8, name=f"o{i}")
                segs = gemm_segments(i)
                for k, ((c0, c1), ps) in enumerate(zip(segs, half)):
                    w = c1 - c0
                    if (i + k) % 2 == 0:
                        nc.scalar.copy(o_sb[:, c0 - i * P : c0 - i * P + w], ps)
                    else:
                        nc.vector.tensor_copy(
                            o_sb[:, c0 - i * P : c0 - i * P + w], ps
                        )
                lo = segs[0][0]
                # the last row-tiles go out via Pool's software DGE, which
                # bypasses the exclusive HWDGE device during the drain
                eng = nc.gpsimd if i >= 5 else nc.sync
                eng.dma_start(
                    out=out_d[i * P : (i + 1) * P, lo:N],
                    in_=o_sb[:, lo - i * P : N - i * P],
                )

            def gemm_half(i):
                # PSUM regions for row-tile i's kept segments
                if i % 2 == 0:
                    psf = psS.tile([P, N], f32, tag="S", name=f"ops{i}")
                    return [
                        psf[:, c0 : c0 + (c1 - c0)]
                        for c0, c1 in gemm_segments(i)
                    ]
                return [
                    psT.tile([P, 512], f32, tag="T", name=f"opt{i}{k}")[
                        :, 0 : c1 - c0
                    ]
                    for k, (c0, c1) in enumerate(gemm_segments(i))
                ]

            def gemm_seg_fin(i, half, k):
                # finish ct=3 for one segment and stream its output
                (c0, c1), ps = list(zip(gemm_segments(i), half))[k]
                nc.tensor.matmul(
                    ps,
                    mergedT[CT - 1][:, i * P : (i + 1) * P],
                    mergedT[CT - 1][:, c0:c1],
                    start=False,
                    stop=True,
                )
                o_sb = stream.tile(
                    [P, N], bf16, tag="o_sb", bufs=8, name=f"o{i}_{k}"
                )
                w = c1 - c0
                if (i + k) % 2 == 0:
                    nc.scalar.copy(o_sb[:, 0:w], ps)
                else:
                    nc.vector.tensor_copy(o_sb[:, 0:w], ps)
                nc.sync.dma_start(
                    out=out_d[i * P : (i + 1) * P, c0:c1], in_=o_sb[:, 0:w]
                )

            # ---- tail: head-7 phase 2 interleaved with the partial final
            # GEMM.  mergedT[0..2] are ready; ct=3 waits on head 7, but the
            # low-column segments only need the first transpose half, so
            # they finish and stream out while phase-2 groups 4-7 run. ----
            halves = {0: gemm_half(0), 2: gemm_half(2)}
            for i in (0, 2):
                # prefill ct 0..2 WITHOUT the stop flag on ct=2
                for (c0, c1), ps in zip(gemm_segments(i), halves[i]):
                    for ct in range(CT - 1):
                        nc.tensor.matmul(
                            ps,
                            mergedT[ct][:, i * P : (i + 1) * P],
                            mergedT[ct][:, c0:c1],
                            start=(ct == 0),
                            stop=False,
                        )
            for g in range(4):
                phase2_group(H - 1, 2 * g)
                phase2_group(H - 1, 2 * g + 1, pool=psT)
                if g == 1:
                    # merged[0..3] col-block 3 complete -> first half of
                    # mergedT[3] transposes while groups 4-7 run
                    merged_transposes(3, halves=(0,))
                if g == 2:
                    gemm_seg_fin(0, halves[0], 0)
                if g == 3:
                    gemm_seg_fin(2, halves[2], 0)
            merged_transposes(3, halves=(1,))
            for i in (0, 2):
                gemm_seg_fin(i, halves[i], 1)
            for i in (1, 4, 3, 6, 5, 7):
                half = gemm_half(i)
                gemm_mms(i, half, range(CT))
                gemm_out(i, half)

    nc.compile()
    return nc


def _get_nc():
    if "nc" not in _CACHE:
        _CACHE["nc"] = _build_nc()
    return _CACHE["nc"]


def make_in_maps(inputs):
    x = np.asarray(inputs["x"], dtype=np.float32)
    bias = np.asarray(inputs["bias"], dtype=np.float32)
    mask = np.asarray(inputs["mask"])
    Wq = np.asarray(inputs["Wq"], dtype=np.float32)
    bq = np.asarray(inputs["bq"], dtype=np.float32)
    Wk = np.asarray(inputs["Wk"], dtype=np.float32)
    bk = np.asarray(inputs["bk"], dtype=np.float32)
    Wv = np.asarray(inputs["Wv"], dtype=np.float32)
    bv = np.asarray(inputs["bv"], dtype=np.float32)

    wqT = Wq.T.astype(ml_dtypes.bfloat16)
    wkT = Wk.T.astype(ml_dtypes.bfloat16)
    # wv65/bv65: 65-wide head slices; weight col 64 is 0 and bias col 64 is
    # 1, giving each V slice a built-in ones column (softmax row-sums)
    wv65 = np.zeros((C, H * (D + 1)), np.float32)
    bv65 = np.zeros((1, H * (D + 1)), np.float32)
    for h in range(H):
        wv65[:, h * 65 : h * 65 + 64] = Wv.T[:, h * 64 : (h + 1) * 64]
        bv65[0, h * 65 : h * 65 + 64] = bv[h * 64 : (h + 1) * 64]
        bv65[0, h * 65 + 64] = 1.0
    wvT = wv65.astype(ml_dtypes.bfloat16)
    # bqk [P, 2*CT]: col ct = bq block ct, col CT+ct = bk block ct
    bqk = np.concatenate(
        [bq.reshape(CT, P).T, bk.reshape(CT, P).T], axis=1
    ).astype(np.float32)
    bqk = np.ascontiguousarray(bqk)
    bvR = np.ascontiguousarray(bv65).astype(ml_dtypes.bfloat16)

    # B^T[h] = exp((bias[h] + (mask-1)*2^30) / 8).T  (bf16; masked -> 0)
    mneg = (mask.astype(np.float32) - 1.0) * (2.0**30)  # [B, N, N]
    BT_all = np.exp((bias + mneg[:, None]) * 0.125)  # [B, H, N, N]
    BT_all = np.ascontiguousarray(BT_all.transpose(0, 1, 3, 2)).astype(
        ml_dtypes.bfloat16
    )

    in_maps = []
    for b in range(NCORES):
        in_maps.append(
            {
                "wfirst": np.ascontiguousarray(
                    np.concatenate(
                        [wqT[:, :P], wkT[:, :P], x[b].T.astype(ml_dtypes.bfloat16)],
                        axis=1,
                    )
                ),
                "wrest": np.ascontiguousarray(
                    np.concatenate([wqT[:, P:], wkT[:, P:], wvT], axis=1)
                ),
                "bqk": bqk,
                "bv": bvR,
                "BT": BT_all[b],
            }
        )
    return in_maps


def run(inputs, trace=False, **kw):
    """Run the SPMD kernel; returns (output [8,1024,1024], BassKernelResults)."""
    from concourse.bass_utils import run_bass_kernel_spmd

    nc = _get_nc()
    in_maps = make_in_maps(inputs)
    res = run_bass_kernel_spmd(
        nc, in_maps, core_ids=list(range(NCORES)), trace=trace, **kw
    )
    out = np.stack(
        [np.asarray(res.results[i]["out"]).astype(np.float32) for i in range(NCORES)],
        axis=0,
    )
    # device skipped everything below the 128-row block diagonal; mirror
    for i in range(1, 8):
        out[:, i * 128 : (i + 1) * 128, : i * 128] = out[
            :, : i * 128, i * 128 : (i + 1) * 128
        ].transpose(0, 2, 1)
    return out, res


def kernel(**inputs):
    out, _ = run(inputs)
    return out


# revision 65
# speedup vs baseline: 1.0113x; 1.0113x over previous
"""Trainium2 Bass kernel: multi-head attention (Graphormer-style bias+mask)
followed by a node-similarity GEMM (out = merged @ merged^T).

Sharding: pure data-parallel over batch. B=8 batch elements -> 8 NeuronCores,
one batch element per core, no collectives. Each core computes its own
[1024, 1024] output slab.

Design (transposed-scores layout; per-core, batch b fixed):
  Q^T = Wq x^T + bq                      [C, N] f32r (d on partitions)
  K^T = Wk x^T                           bk is softmax-shift-invariant: the
                                         b_k.Q[n] and b_k.b_q score terms are
                                         constant along the softmax axis, so
                                         dropping bk is mathematically exact
  V   = x Wv^T + bv                      [N, C] bf16, head-slices computed
                                         just in time (one slice per step)
  S^T[m,n] = K Q^T                       per (head, m-tile): lhsT=K^T-slice,
                                         rhs=Q^T -> PSUM [128, N]; no bias or
                                         mask work on the PE at all
  E0  = exp(S^T/8)                       ACT, psum -> sbuf bf16 (the pacing
                                         engine: 64 x [128,1024] exps)
  E^T = E0 * B^T[h]                      DVE 2x / Pool; B = exp((bias+mneg)/8)
                                         folded on the host (exp(a+b) =
                                         exp(a)exp(b)), so masked entries are
                                         exactly 0 and no mask add is needed
  A[n,dslice], rs[n] = E^T-blocks @ V    A-natural matmuls: lhsT=E^T-block
                                         (m on partitions), rhs=V-slice/ones;
                                         rowsums ride along as a free column
  merged[n, h*64:..] = A * (1/rs)        DVE normalize fused into the
                                         PSUM->SBUF copy
  mergedT = transpose(merged)            PE bf16 transposes per head-pair
  out = mergedT^T @ mergedT              f32 PSUM, bf16 operands; blocks
                                         below the 128-row block diagonal are
                                         skipped (symmetry) and mirrored on
                                         the host

Schedule: a software pipeline interleaves, per (head, m-tile) step, the S^T
matmuls, the lagged phase-2 (A+normalize) groups of the previous head, the
JIT V slices, spread-out Q^T/K^T blocks, and merged transposes, so the PE
stream never blocks the ACT exp stream.  E^T double-buffers by head parity.
The final GEMM prefills its first PSUM accumulation groups (contractions over
finished channel blocks) inside the last head's phase-2 window.
"""

import sys

if "/opt/trn_rl_repo" not in sys.path:
    sys.path.insert(0, "/opt/trn_rl_repo")

import ml_dtypes
import numpy as np

P = 128
N = 1024
C = 512
H = 8
D = 64  # head dim
NT = N // P  # 8 row tiles
CT = C // P  # 4 channel tiles
NCORES = 8

_CACHE = {}


def _build_nc():
    import concourse.mybir as mybir
    import concourse.tile as tile
    from concourse import bacc
    from concourse.masks import make_identity

    f32 = mybir.dt.float32
    f32r = mybir.dt.float32r
    bf16 = mybir.dt.bfloat16
    Act = mybir.ActivationFunctionType

    nc = bacc.Bacc("TRN2", target_bir_lowering=False, debug=False)

    # ---- DRAM parameters (per-core) ----
    # wfirst rows = input channel; cols = [wq strip ct0 | wk strip ct0 | x^T]
    # wrest cols = [wq strips ct1-3 | wk strips ct1-3 | wv]  (all bf16)
    WF = 2 * P + N
    D65 = D + 1
    WR = 2 * (C - P) + H * D65
    wfirst_d = nc.dram_tensor("wfirst", [C, WF], bf16, kind="ExternalInput")
    wrest_d = nc.dram_tensor("wrest", [C, WR], bf16, kind="ExternalInput")
    bqk_d = nc.dram_tensor("bqk", [P, 2 * CT], f32, kind="ExternalInput")
    bv_d = nc.dram_tensor("bv", [1, H * (D + 1)], bf16, kind="ExternalInput")
    BT_d = nc.dram_tensor("BT", [H, N, N], bf16, kind="ExternalInput")
    out_d = nc.dram_tensor("out", [N, N], bf16, kind="ExternalOutput")

    with tile.TileContext(nc) as tc:
        with (
            tc.tile_pool(name="const", bufs=1) as constp,
            tc.tile_pool(name="pers", bufs=1) as pers,
            tc.tile_pool(name="stream", bufs=2) as stream,
            tc.tile_pool(name="psS", bufs=2, space="PSUM") as psS,
            tc.tile_pool(name="psA", bufs=2, space="PSUM") as psA,
            tc.tile_pool(name="psT", bufs=2, space="PSUM") as psT,
        ):
            ident = constp.tile([P, P], f32)
            make_identity(nc, ident[:])
            ident_b = constp.tile([P, P], bf16)
            nc.vector.tensor_copy(ident_b[:], ident[:])

            warm = constp.tile([P, 1], f32)
            nc.scalar.activation(warm[:], ident[:, 0:1], Act.Exp, scale=1.0)
            # touch the PE right away so its p-state ramp timer starts
            # counting from ~0 and the first QK matmuls run at full clock
            warm_ps = psA.tile([1, 1], f32, tag="A", name="warm_ps")
            nc.tensor.matmul(
                warm_ps[:], ident[:, 0:1], ident[:, 0:1], start=True, stop=True
            )

            # ---- persistent SBUF tensors ----
            QT = [pers.tile([P, N], f32r, name=f"QT{i}") for i in range(CT)]
            KT = [pers.tile([P, N], f32r, name=f"KT{i}") for i in range(CT)]
            V = [pers.tile([P, H * D65], bf16, name=f"V{i}") for i in range(NT)]
            # E^T tiles, double-buffered by head parity: [slot][m-tile]
            ET = [
                [pers.tile([P, N], bf16, name=f"ET{s}_{i}") for i in range(NT)]
                for s in range(2)
            ]
            merged = [pers.tile([P, C], bf16, name=f"merged{i}") for i in range(NT)]
            mergedT = [pers.tile([P, N], bf16, name=f"mergedT{i}") for i in range(CT)]
            bqk_sb = pers.tile([P, 2 * CT], f32, name="bqk_sb")
            bv_sb = pers.tile([1, H * D65], bf16, name="bv_sb")
            ones_b = pers.tile([1, N], bf16, name="ones_b")
            wfirst = [pers.tile([P, WF], bf16, name=f"wfirst{i}") for i in range(CT)]
            wrest = [pers.tile([P, WR], bf16, name=f"wrest{i}") for i in range(CT)]
            xTb = [wfirst[i][:, 2 * P : WF] for i in range(CT)]
            wv = [wrest[i][:, 2 * (C - P) : WR] for i in range(CT)]

            def wq_strip(kt, ct):
                if ct == 0:
                    return wfirst[kt][:, 0:P]
                return wrest[kt][:, (ct - 1) * P : ct * P]

            def wk_strip(kt, ct):
                if ct == 0:
                    return wfirst[kt][:, P : 2 * P]
                return wrest[kt][:, (C - P) + (ct - 1) * P : (C - P) + ct * P]

            SPLIT = 2 * P + 512  # [wq0 | wk0 | x cols 0:512]
            for i in range(CT):
                nc.sync.dma_start(
                    out=wfirst[i][:, 0:SPLIT],
                    in_=wfirst_d[i * P : (i + 1) * P, 0:SPLIT],
                )
            nc.sync.dma_start(out=bqk_sb[:], in_=bqk_d[:])

            def qk_chunk(ct, unit):
                """One (w, j) quarter of Q^T/K^T rows ct*128..: 4 matmuls."""
                strip, dst, boff = ((wq_strip, QT, 0), (wk_strip, KT, CT))[unit // 2]
                j = unit % 2
                ps = psT.tile([P, 512], f32, tag="T", name=f"qk{ct}{unit}")
                for kt in range(CT):
                    nc.tensor.matmul(
                        ps[:],
                        strip(kt, ct),
                        xTb[kt][:, j * 512 : (j + 1) * 512],
                        start=(kt == 0),
                        stop=(kt == CT - 1),
                    )
                if unit // 2 == 0:
                    nc.vector.tensor_scalar_add(
                        dst[ct][:, j * 512 : (j + 1) * 512],
                        ps[:],
                        bqk_sb[:, boff + ct : boff + ct + 1],
                    )
                else:
                    # bk is softmax-shift-invariant -> plain copy
                    nc.vector.tensor_copy(
                        dst[ct][:, j * 512 : (j + 1) * 512], ps[:]
                    )

            # Q^T/K^T block 0, j half at a time, kt round-robin so the
            # matmuls track the wfirst DMA arrivals; the first S^T tile and
            # its exp run per half so ACT starts as early as possible.
            def qk_block0_half(j):
                pss = {}
                for unit in (j, 2 + j):
                    pss[unit] = psT.tile(
                        [P, 512], f32, tag="T", name=f"qk0{unit}"
                    )
                for kt in range(CT):
                    for unit in (j, 2 + j):
                        strip = (wq_strip, wk_strip)[unit // 2]
                        nc.tensor.matmul(
                            pss[unit][:],
                            strip(kt, 0),
                            xTb[kt][:, j * 512 : (j + 1) * 512],
                            start=(kt == 0),
                            stop=(kt == CT - 1),
                        )
                nc.vector.tensor_scalar_add(
                    QT[0][:, j * 512 : (j + 1) * 512],
                    pss[j][:],
                    bqk_sb[:, 0:1],
                )
                nc.vector.tensor_copy(
                    KT[0][:, j * 512 : (j + 1) * 512], pss[2 + j][:]
                )

            nc.vector.memset(ones_b[:], 1.0)

            def late_input_dmas():
                for i in range(CT):
                    nc.sync.dma_start(
                        out=wrest[i][:], in_=wrest_d[i * P : (i + 1) * P, :]
                    )
                nc.sync.dma_start(out=bv_sb[:], in_=bv_d[:])

            def v_slice(h, mt):
                # V65[mt][:, h*65:(h+1)*65] = (x Wv65^T + bv65) head-slice,
                # JIT.  wv65 col 64 of each slice is 0 and bv65 col 64 is 1,
                # so the slice carries a built-in ones column that later
                # yields the softmax row-sums for free in the A-matmul.
                ps = psA.tile([P, 72], f32, tag="A", name=f"vps{h}{mt}")
                for kt in range(CT):
                    nc.tensor.matmul(
                        ps[:, 0:D65],
                        xTb[kt][:, mt * P : (mt + 1) * P],
                        wv[kt][:, h * D65 : (h + 1) * D65],
                        start=(kt == 0),
                        stop=False,
                    )
                nc.tensor.matmul(
                    ps[:, 0:D65],
                    ones_b[:, mt * P : (mt + 1) * P],
                    bv_sb[:, h * D65 : (h + 1) * D65],
                    start=False,
                    stop=True,
                )
                nc.vector.tensor_copy(
                    V[mt][:, h * D65 : (h + 1) * D65], ps[:, 0:D65]
                )

            # ---- main loop over heads (software-pipelined) ----
            # Iteration h emits phase 1 (S^T -> E^T) of head h interleaved
            # with phase 2 (A-natural + normalize) of head h-1, so the PE
            # always has ready work while ACT chews through the exps.
            st_tiles = {}

            def st_tile(h, mt):
                qt = QT[h // 2]
                kt_sb = KT[h // 2]
                po = (h % 2) * D
                bt = stream.tile([P, N], bf16, tag="bt", bufs=6, name=f"bt{h}{mt}")
                nc.sync.dma_start(out=bt[:], in_=BT_d[h, mt * P : (mt + 1) * P, :])
                ST = psS.tile([P, N], f32, tag="S", name=f"ST{h}{mt}")
                for j in range(2):
                    nc.tensor.matmul(
                        ST[:, j * 512 : (j + 1) * 512],
                        kt_sb[po : po + D, mt * P : (mt + 1) * P],
                        qt[po : po + D, j * 512 : (j + 1) * 512],
                        start=True,
                        stop=True,
                    )
                st_tiles[(h, mt)] = (ST, bt)

            def ex_tile(h, mt, split=False):
                ST, bt = st_tiles.pop((h, mt))
                s = h % 2
                e0 = stream.tile([P, N], bf16, tag="e0", bufs=4, name=f"e0{h}{mt}")
                if split:
                    # first tile: exp each half as soon as its S^T half is
                    # ready, so ACT starts ~1.7us earlier
                    for j in range(2):
                        nc.scalar.activation(
                            e0[:, j * 512 : (j + 1) * 512],
                            ST[:, j * 512 : (j + 1) * 512],
                            Act.Exp,
                            scale=0.125,
                        )
                else:
                    nc.scalar.activation(e0[:], ST[:], Act.Exp, scale=0.125)
                # E^T = E0 * B^T (masked entries have B == 0); all-bf16
                # packed operands -> DVE 2x_1p mode.  Three tiles per head
                # go to the otherwise-idle Pool engine to unload DVE.
                eng = nc.gpsimd if mt in (0, 3, 6) else nc.vector
                eng.tensor_mul(ET[s][mt][:], e0[:], bt[:])

            def phase2_group(h, i, pool=None, trailing=False):
                s = h % 2
                Aps = (pool or psA).tile(
                    [P, 72], f32, tag="A" if pool is None else "T", name=f"A{h}{i}"
                )
                for mt in range(NT):
                    nc.tensor.matmul(
                        Aps[:, 0:D65],
                        ET[s][mt][:, i * P : (i + 1) * P],
                        V[mt][:, h * D65 : (h + 1) * D65],
                        start=(mt == 0),
                        stop=(mt == NT - 1),
                    )
                # merged = A * (1/rowsum)  (normalize while copying out)
                rc = stream.tile([P, 1], f32, tag="rc", bufs=4, name=f"rc{h}{i}")
                nc.vector.reciprocal(rc[:], Aps[:, D : D + 1])
                nc.vector.tensor_scalar_mul(
                    merged[i][:, h * D : (h + 1) * D], Aps[:, 0:D], rc[:]
                )

            def merged_transposes(ct, halves=(0, 1)):
                for half in halves:
                    tp = psT.tile([P, 512], bf16, tag="T", name=f"tp{ct}{half}")
                    for q in range(4):
                        i = half * 4 + q
                        nc.tensor.transpose(
                            tp[:, q * P : (q + 1) * P],
                            merged[i][:, ct * P : (ct + 1) * P],
                            ident_b[:],
                        )
                    nc.vector.tensor_copy(
                        mergedT[ct][:, half * 512 : (half + 1) * 512], tp[:]
                    )

            # Main loop: per (head, m-tile) step emit the S^T matmuls
            # first, then the lagged phase-2 group of the previous head,
            # then exp/B-mult (ACT only ever waits on the S^T matmuls,
            # which execute before the phase-2 burst), then side jobs.
            # hand-emitted warm-up: head 0 tile 0.  Both qk half-blocks are
            # emitted before the first S^T matmul so the half-1 matmuls can
            # track the x-j1 DMA arrivals instead of head-of-line blocking
            # behind ST00-j0 (which waits on the DVE bias add).
            qk_block0_half(0)
            bt00 = stream.tile([P, N], bf16, tag="bt", bufs=6, name="bt00")
            nc.sync.dma_start(out=bt00[:], in_=BT_d[0, 0:P, :])
            ST00 = psS.tile([P, N], f32, tag="S", name="ST00")
            e000 = stream.tile([P, N], bf16, tag="e0", bufs=4, name="e000")
            for i in range(CT):
                nc.sync.dma_start(
                    out=wfirst[i][:, SPLIT:WF],
                    in_=wfirst_d[i * P : (i + 1) * P, SPLIT:WF],
                )
            qk_block0_half(1)
            for j in range(2):
                nc.tensor.matmul(
                    ST00[:, j * 512 : (j + 1) * 512],
                    KT[0][0:D, 0:P],
                    QT[0][0:D, j * 512 : (j + 1) * 512],
                    start=True,
                    stop=True,
                )
                nc.scalar.activation(
                    e000[:, j * 512 : (j + 1) * 512],
                    ST00[:, j * 512 : (j + 1) * 512],
                    Act.Exp,
                    scale=0.125,
                )
            nc.vector.tensor_mul(ET[0][0][:], e000[:], bt00[:])

            for h in range(H):
                for mt in range(NT):
                    if h == 0 and mt == 0:
                        continue
                    st_tile(h, mt)
                    if h > 0 and mt >= 2:
                        # 2-tile lag so ET[h-1] is surely complete
                        phase2_group(h - 1, mt - 2)
                    if h > 0 and mt == 0:
                        phase2_group(h - 1, 6, trailing=True)
                    if h > 0 and mt == 1:
                        phase2_group(h - 1, 7, trailing=True)
                    if h >= 3 and h % 2 == 1 and mt == 2:
                        merged_transposes((h - 3) // 2)
                    ex_tile(h, mt)
                    if h == 0:
                        # head-0: late-input DMAs at mt 1, V slices after
                        # wrest lands, qk block 1 spread over mt 5..7
                        if mt == 1:
                            late_input_dmas()
                        if mt >= 4:
                            v_slice(0, 2 * (mt - 4))
                            v_slice(0, 2 * (mt - 4) + 1)
                        if mt >= 5:
                            qk_chunk(1, mt - 5)
                    else:
                        v_slice(h, mt)
                        # remaining Q^T/K^T chunks, at most two per head so
                        # no head's PE budget exceeds the ACT exp period
                        QK_SCHED = {
                            (1, 1): (1, 3),
                            (1, 5): (2, 0),
                            (2, 1): (2, 1),
                            (2, 5): (2, 2),
                            (3, 1): (2, 3),
                            (3, 5): (3, 0),
                            (4, 1): (3, 1),
                            (4, 5): (3, 2),
                            (5, 1): (3, 3),
                        }
                        if (h, mt) in QK_SCHED:
                            qk_chunk(*QK_SCHED[(h, mt)])


            def gemm_segments(i):
                # kept output columns for row-tile i: the diagonal and above
                # (out is symmetric; the host mirrors the rest)
                c0 = i * P
                return [(c0, 512), (512, 1024)] if c0 < 512 else [(c0, 1024)]

            def gemm_mms(i, half, cts):
                for (c0, c1), ps in zip(gemm_segments(i), half):
                    for ct in cts:
                        nc.tensor.matmul(
                            ps,
                            mergedT[ct][:, i * P : (i + 1) * P],
                            mergedT[ct][:, c0:c1],
                            start=(ct == 0),
                            stop=(ct == CT - 1),
                        )

            def gemm_out(i, half):
                # copies per segment (ACT/DVE alternating), then a single
                # DMA covering the whole kept column range (fewer HWDGE
                # round-trips in the drain)
                o_sb = stream.tile([P, N], bf16, tag="o_sb", bufs=8, name=f"o{i}")
                segs = gemm_segments(i)
                for k, ((c0, c1), ps) in enumerate(zip(segs, half)):
                    w = c1 - c0
                    if (i + k) % 2 == 0:
                        nc.scalar.copy(o_sb[:, c0 - i * P : c0 - i * P + w], ps)
                    else:
                        nc.vector.tensor_copy(
                            o_sb[:, c0 - i * P : c0 - i * P + w], ps
                        )
                lo = segs[0][0]
                nc.sync.dma_start(
                    out=out_d[i * P : (i + 1) * P, lo:N],
                    in_=o_sb[:, lo - i * P : N - i * P],
                )

            def gemm_half(i):
                # PSUM regions for row-tile i's kept segments
                if i % 2 == 0:
                    psf = psS.tile([P, N], f32, tag="S", name=f"ops{i}")
                    return [
                        psf[:, c0 : c0 + (c1 - c0)]
                        for c0, c1 in gemm_segments(i)
                    ]
                return [
                    psT.tile([P, 512], f32, tag="T", name=f"opt{i}{k}")[
                        :, 0 : c1 - c0
                    ]
                    for k, (c0, c1) in enumerate(gemm_segments(i))
                ]

            def gemm_seg_fin(i, half, k):
                # finish ct=3 for one segment and stream its output
                (c0, c1), ps = list(zip(gemm_segments(i), half))[k]
                nc.tensor.matmul(
                    ps,
                    mergedT[CT - 1][:, i * P : (i + 1) * P],
                    mergedT[CT - 1][:, c0:c1],
                    start=False,
                    stop=True,
                )
                o_sb = stream.tile(
                    [P, N], bf16, tag="o_sb", bufs=8, name=f"o{i}_{k}"
                )
                w = c1 - c0
                if (i + k) % 2 == 0:
                    nc.scalar.copy(o_sb[:, 0:w], ps)
                else:
                    nc.vector.tensor_copy(o_sb[:, 0:w], ps)
                nc.sync.dma_start(
                    out=out_d[i * P : (i + 1) * P, c0:c1], in_=o_sb[:, 0:w]
                )

            # ---- tail: head-7 phase 2 interleaved with the partial final
            # GEMM.  mergedT[0..2] are ready; ct=3 waits on head 7, but the
            # low-column segments only need the first transpose half, so
            # they finish and stream out while phase-2 groups 4-7 run. ----
            halves = {0: gemm_half(0), 2: gemm_half(2)}
            for i in (0, 2):
                # prefill ct 0..2 WITHOUT the stop flag on ct=2
                for (c0, c1), ps in zip(gemm_segments(i), halves[i]):
                    for ct in range(CT - 1):
                        nc.tensor.matmul(
                            ps,
                            mergedT[ct][:, i * P : (i + 1) * P],
                            mergedT[ct][:, c0:c1],
                            start=(ct == 0),
                            stop=False,
                        )
            for g in range(4):
                phase2_group(H - 1, 2 * g)
                phase2_group(H - 1, 2 * g + 1, pool=psT)
                if g == 1:
                    # merged[0..3] col-block 3 complete -> first half of
                    # mergedT[3] transposes while groups 4-7 run
                    merged_transposes(3, halves=(0,))
                if g == 2:
                    gemm_seg_fin(0, halves[0], 0)
                if g == 3:
                    gemm_seg_fin(2, halves[2], 0)
            merged_transposes(3, halves=(1,))
            for i in (0, 2):
                gemm_seg_fin(i, halves[i], 1)
            for i in (1, 4, 3, 6, 5, 7):
                half = gemm_half(i)
                gemm_mms(i, half, range(CT))
                gemm_out(i, half)

    nc.compile()
    return nc


def _get_nc():
    if "nc" not in _CACHE:
        _CACHE["nc"] = _build_nc()
    return _CACHE["nc"]


def make_in_maps(inputs):
    x = np.asarray(inputs["x"], dtype=np.float32)
    bias = np.asarray(inputs["bias"], dtype=np.float32)
    mask = np.asarray(inputs["mask"])
    Wq = np.asarray(inputs["Wq"], dtype=np.float32)
    bq = np.asarray(inputs["bq"], dtype=np.float32)
    Wk = np.asarray(inputs["Wk"], dtype=np.float32)
    bk = np.asarray(inputs["bk"], dtype=np.float32)
    Wv = np.asarray(inputs["Wv"], dtype=np.float32)
    bv = np.asarray(inputs["bv"], dtype=np.float32)

    wqT = Wq.T.astype(ml_dtypes.bfloat16)
    wkT = Wk.T.astype(ml_dtypes.bfloat16)
    # wv65/bv65: 65-wide head slices; weight col 64 is 0 and bias col 64 is
    # 1, giving each V slice a built-in ones column (softmax row-sums)
    wv65 = np.zeros((C, H * (D + 1)), np.float32)
    bv65 = np.zeros((1, H * (D + 1)), np.float32)
    for h in range(H):
        wv65[:, h * 65 : h * 65 + 64] = Wv.T[:, h * 64 : (h + 1) * 64]
        bv65[0, h * 65 : h * 65 + 64] = bv[h * 64 : (h + 1) * 64]
        bv65[0, h * 65 + 64] = 1.0
    wvT = wv65.astype(ml_dtypes.bfloat16)
    # bqk [P, 2*CT]: col ct = bq block ct, col CT+ct = bk block ct
    bqk = np.concatenate(
        [bq.reshape(CT, P).T, bk.reshape(CT, P).T], axis=1
    ).astype(np.float32)
    bqk = np.ascontiguousarray(bqk)
    bvR = np.ascontiguousarray(bv65).astype(ml_dtypes.bfloat16)

    # B^T[h] = exp((bias[h] + (mask-1)*2^30) / 8).T  (bf16; masked -> 0)
    mneg = (mask.astype(np.float32) - 1.0) * (2.0**30)  # [B, N, N]
    BT_all = np.exp((bias + mneg[:, None]) * 0.125)  # [B, H, N, N]
    BT_all = np.ascontiguousarray(BT_all.transpose(0, 1, 3, 2)).astype(
        ml_dtypes.bfloat16
    )

    in_maps = []
    for b in range(NCORES):
        in_maps.append(
            {
                "wfirst": np.ascontiguousarray(
                    np.concatenate(
                        [wqT[:, :P], wkT[:, :P], x[b].T.astype(ml_dtypes.bfloat16)],
                        axis=1,
                    )
                ),
                "wrest": np.ascontiguousarray(
                    np.concatenate([wqT[:, P:], wkT[:, P:], wvT], axis=1)
                ),
                "bqk": bqk,
                "bv": bvR,
                "BT": BT_all[b],
            }
        )
    return in_maps


def run(inputs, trace=False, **kw):
    """Run the SPMD kernel; returns (output [8,1024,1024], BassKernelResults)."""
    from concourse.bass_utils import run_bass_kernel_spmd

    nc = _get_nc()
    in_maps = make_in_maps(inputs)
    res = run_bass_kernel_spmd(
        nc, in_maps, core_ids=list(range(NCORES)), trace=trace, **kw
    )
    out = np.stack(
        [np.asarray(res.results[i]["out"]).astype(np.float32) for i in range(NCORES)],
        axis=0,
    )
    # device skipped everything below the 128-row block diagonal; mirror
    for i in range(1, 8):
        out[:, i * 128 : (i + 1) * 128, : i * 128] = out[
            :, : i * 128, i * 128 : (i + 1) * 128
        ].transpose(0, 2, 1)
    return out, res


def kernel(**inputs):
    out, _ = run(inputs)
    return out
